# revision 36
# baseline (speedup 1.0000x reference)
"""EvolveGCNO RecurrentGCN forward on 8 trn2 NeuronCores.

Strategy (dst-sharded gather, bf16, balanced layout):
  - Nodes sharded by destination across 8 cores (6250 each, padded to 6400).
    Within each core, nodes are RELABELED by a host-side balanced
    partitioning (_balance_layout): sources are 2-colored into table halves
    and destinations greedily packed into 128-dst windows so every
    (core, window, src-half) edge group fits in K=6 chunks of 128 edges
    (~2% gather padding instead of ~20% for the natural layout).
  - Phase A (device): per-core degree via padded-CSR row sums (bf16), dinv =
    1/sqrt(deg + 1); scale own x rows by dinv -> xs (bf16); ONE AllGather of
    xs laid out as [3200, 2C] row-pairs (table row r = local rows r and
    r+3200), so the full 25600-row table stays int16-indexable and gathers
    select the pair half via a column-offset access pattern with 2C row
    pitch. x arrives pre-paired in bf16 so every head DMA run is 512B
    (avoids the <512B descriptor penalty).
  - GRU weight evolution on device (replicated; schedule-deferred off the
    phase-A critical path).
  - Phase B (device): per block of up to 512 dsts, dma_gather bf16 xs[src]
    rows (256B descriptors - the memory-roofline term, ~77k rows/core);
    build one-hot scatter matrices S[e, j] = (iota == off) * ew in bf16
    (DVE 2-byte fast path, 94ns) per 128-edge chunk; aggregate with bf16 PE
    matmuls into PSUM [128f, 512d] (Ldweights is free, cost keyed on the
    moving operand); self-loops enter as identity matmuls on local xs
    tiles; then W matmul, ReLU, lin row per window; dinv[dst] commutes past
    ReLU/lin and is applied as a per-partition [128, 1] column op. Output
    is stored partition-major [128, 50]; the host inverts the relabeling.

Host work is limited to graph partitioning / index manipulation / layout
(sorting, bincount, padding, dtype formatting, parameter transposes); all
floating point math on tensor values happens on device.
"""

import math
import sys

import numpy as np

sys.path.insert(0, "/opt/trn_rl_repo")

N_NODES, N_EDGES, C = 50000, 600000, 128
NCORES = 8
NPC = N_NODES // NCORES            # 6250 nodes per core
NTILE = 50                         # sbuf tiles of 128 nodes
NPAD = NTILE * 128                 # 6400 padded nodes per core
HALFL = NPAD // 2                  # 3200: per-core pair split
TROWS = NCORES * HALFL             # 25600 rows in the paired gather table
WDST = 128                         # dsts per psum column window
NWINDOW = NPAD // WDST             # 50 windows per core
BLK = 512                          # dsts per psum block
WPB = BLK // WDST                  # full-block windows (4)
NBLK = 14                          # 12 full blocks + two 1-window blocks


# ---------------------------------------------------------------------------
# Host-side preprocessing: graph partitioning + layout (index work only)
# ---------------------------------------------------------------------------

def _balance_layout(src: np.ndarray, dst: np.ndarray):
    """Assign each node a local row l in [0, NPAD) on its owning core so that
    every (core, window, src-half) edge group stays under 6*128 edges:
    sources are 2-colored (half = l // HALFL, i.e. window < NWINDOW/2), and
    destinations are packed into windows by greedy balanced partitioning of
    in-degree counts (with a secondary term canceling the src-color signs).
    Pure index manipulation. Returns L[node] -> local row."""
    # pass A: color srcs by parity of their id (balanced per core: 3125/3125)
    H = (np.arange(N_NODES) % 2).astype(np.int64)
    # per-dst in-edge counts split by src color
    lo_in = np.bincount(dst[H[src] == 0], minlength=N_NODES)
    hi_in = np.bincount(dst[H[src] == 1], minlength=N_NODES)
    L = np.zeros(N_NODES, np.int64)
    nwh = NWINDOW // 2
    for c in range(NCORES):
        own = np.arange(c * NPC, (c + 1) * NPC)
        for h in (0, 1):
            nodes = own[H[own] == h]
            deg = lo_in[nodes] + hi_in[nodes]
            order = nodes[np.argsort(-deg, kind="stable")]
            llo = np.zeros(nwh, np.int64)
            lhi = np.zeros(nwh, np.int64)
            counts = np.zeros(nwh, np.int64)
            for v in order:
                # minimize the larger per-half group count directly
                cost = (np.maximum(llo + lo_in[v], lhi + hi_in[v]) * 4096
                        + llo + lo_in[v] + lhi + hi_in[v]
                        + (counts >= 128) * (1 << 50))
                w = int(np.argmin(cost))
                L[v] = (h * nwh + w) * WDST + counts[w]
                counts[w] += 1
                llo[w] += lo_in[v]
                lhi[w] += hi_in[v]
    return L


def preprocess(edge_index: np.ndarray, edge_weight: np.ndarray):
    src = np.asarray(edge_index[0], dtype=np.int64)
    dst = np.asarray(edge_index[1], dtype=np.int64)
    ew = np.asarray(edge_weight, dtype=np.float32)

    L = _balance_layout(src, dst)
    core_of = dst // NPC
    nw_b = [4] * 12 + [1, 1]
    percore = []
    kl = kh = 1
    kdeg = 1
    for c in range(NCORES):
        m = core_of == c
        s = src[m]
        l = L[dst[m]]
        w = ew[m]
        sc = s // NPC
        sl = L[s]
        half = (sl >= HALFL).astype(np.int64)
        idx16 = (sc * HALFL + (sl - half * HALFL)).astype(np.int64)
        # group edges by (window, half), dst-sorted within each group
        key = (l // WDST) * 2 + half
        order = np.argsort(key * (NPAD + 1) + l, kind="stable")
        l, w, half, idx16 = l[order], w[order], half[order], idx16[order]
        win = l // WDST
        percore.append((l, w, half, idx16, win))
        for h in (0, 1):
            cnt = np.bincount(win[half == h], minlength=NWINDOW)
            k = int(math.ceil(max(int(cnt.max()), 1) / 128))
            if h == 0:
                kl = max(kl, k)
            else:
                kh = max(kh, k)
        kdeg = max(kdeg, int(np.bincount(l, minlength=NPAD).max()))

    KL, KH, KDEG = kl, kh, kdeg
    blk_start = np.cumsum([0] + [nw * (KL + KH) for nw in nw_b])
    TOT = int(blk_start[-1])                       # total meta columns

    lo_starts, hi_starts = [], []
    lo_c = hi_c = 0
    for bb in range(NBLK):
        nwb = nw_b[bb]
        lo_starts.append(lo_c)
        hi_starts.append(hi_c)
        lo_c += nwb * KL * 8
        hi_c += nwb * KH * 8
    CL, CH = lo_c, hi_c

    metas = []
    for c in range(NCORES):
        l, w, half, idx16, win = percore[c]
        wb_base = np.cumsum([0] + nw_b[:-1])
        b = np.searchsorted(np.cumsum(nw_b), win, side="right")
        wb = win - wb_base[b]                       # window within block
        nw = np.array(nw_b)[b]
        # position within the (window, half) group
        grp = win * 2 + half
        gcnt = np.bincount(grp, minlength=2 * NWINDOW)
        gstart = np.cumsum(gcnt) - gcnt
        p_in = np.arange(len(l)) - gstart[grp]
        j = p_in // 128
        row = p_in % 128
        col = np.where(
            half == 0,
            blk_start[b] + wb * KL + j,
            blk_start[b] + nw * KL + wb * KH + j,
        )

        offv = np.zeros((128, TOT), np.float32)
        ewv = np.zeros((128, TOT), np.float32)
        offv[row, col] = (l % WDST).astype(np.float32)
        ewv[row, col] = w

        # gather index lists, one per (block, half), packed along columns.
        # list position i = (col_rel * 128 + row); idx 0 pads (killed by ew=0)
        idxlo = np.zeros((16, CL), np.int16)
        idxhi = np.zeros((16, CH), np.int16)
        for h, (arr, starts, KX) in enumerate(
                [(idxlo, lo_starts, KL), (idxhi, hi_starts, KH)]):
            mh = half == h
            bb = b[mh]
            col_rel = wb[mh] * KX + j[mh]           # position within half
            i_list = col_rel * 128 + row[mh]        # position in block's list
            ci = np.array(starts)[bb] * 16 + i_list  # global flat position
            arr[ci % 16, ci // 16] = idx16[mh]
        metas.append(dict(
            offv=offv,
            ewv=ewv,
            idxlo=np.ascontiguousarray(np.tile(idxlo, (8, 1))),
            idxhi=np.ascontiguousarray(np.tile(idxhi, (8, 1))),
        ))

    # padded CSR of edge weights for the degree computation (no self-loops;
    # the +1 self weight is added on device via the sqrt bias)
    for c in range(NCORES):
        l, w, half, idx16, win = percore[c]
        counts = np.bincount(l, minlength=NPAD)
        starts = np.cumsum(counts) - counts
        o2 = np.argsort(l, kind="stable")
        ls, ws = l[o2], w[o2]
        slot = np.arange(len(ls)) - starts[ls]
        csr = np.zeros((NPAD, KDEG), np.float32)
        csr[ls, slot] = ws
        metas[c]["csr"] = _to_bf16(csr)

    pre = dict(KL=KL, KH=KH, KDEG=KDEG, TOT=TOT, nw_b=nw_b,
               blk_start=[int(v) for v in blk_start],
               lo_starts=lo_starts, hi_starts=hi_starts, CL=CL, CH=CH)
    return pre, metas, L


def _to_bf16(a: np.ndarray) -> np.ndarray:
    import ml_dtypes
    return np.ascontiguousarray(a.astype(ml_dtypes.bfloat16))


def make_in_maps(inp: dict, pre, metas, L):
    iota = np.tile(np.arange(WDST, dtype=np.float32), (128, 1))
    ident = np.eye(128, dtype=np.float32)
    W0 = np.asarray(inp["W0"], np.float32)
    x = np.ascontiguousarray(np.asarray(inp["x"], np.float32))
    shared = dict(
        iota=_to_bf16(iota),
        ident=_to_bf16(ident),
        w0=W0,
        w0t=np.ascontiguousarray(W0.T),
        wiht=np.ascontiguousarray(np.asarray(inp["gru_w_ih"], np.float32).T),
        whht=np.ascontiguousarray(np.asarray(inp["gru_w_hh"], np.float32).T),
        bih=np.asarray(inp["gru_b_ih"], np.float32),
        bhh=np.asarray(inp["gru_b_hh"], np.float32),
        linw=np.ascontiguousarray(np.asarray(inp["lin_w"], np.float32).T),
        linb=np.asarray(inp["lin_b"], np.float32).reshape(1, 1),
    )
    maps = []
    for c in range(NCORES):
        xo = np.zeros((NPAD, C), np.float32)
        own = np.arange(c * NPC, (c + 1) * NPC)
        xo[L[own]] = x[own]
        x2 = np.concatenate([xo[:HALFL], xo[HALFL:]], axis=1)
        m = dict(shared, x_own=_to_bf16(x2),
                 offv=metas[c]["offv"], ewv=metas[c]["ewv"],
                 idxlo=metas[c]["idxlo"], idxhi=metas[c]["idxhi"],
                 csr=metas[c]["csr"])
        maps.append(m)
    return maps


# ---------------------------------------------------------------------------
# Device program
# ---------------------------------------------------------------------------

def build_program(pre, skip_collective: bool = False, nblk: int = NBLK,
                  rep: int = 1, nqueues: int = 4):
    import concourse.bacc as bacc
    import concourse.bass as bass
    import concourse.tile as tile
    from concourse import mybir

    f32 = mybir.dt.float32
    bf16 = mybir.dt.bfloat16
    i16 = mybir.dt.int16
    AF = mybir.ActivationFunctionType
    OP = mybir.AluOpType
    KL, KH, KDEG, TOT = pre["KL"], pre["KH"], pre["KDEG"], pre["TOT"]
    nw_b = pre["nw_b"]
    blk_start = pre["blk_start"]
    lo_starts, hi_starts = pre["lo_starts"], pre["hi_starts"]
    CL, CH = pre["CL"], pre["CH"]
    MAXCOL = WPB * (KL + KH)          # widest block in meta columns

    nc = bacc.Bacc("TRN2", target_bir_lowering=False, debug=False,
                   num_devices=NCORES, num_swdge_queues=nqueues)

    x_own_t = nc.declare_dram_parameter("x_own", [HALFL, 2 * C], bf16,
                                       isOutput=False)
    idxlo_t = nc.declare_dram_parameter("idxlo", [128, CL], i16, isOutput=False)
    idxhi_t = nc.declare_dram_parameter("idxhi", [128, CH], i16, isOutput=False)
    offv_t = nc.declare_dram_parameter("offv", [128, TOT], f32, isOutput=False)
    ewv_t = nc.declare_dram_parameter("ewv", [128, TOT], f32, isOutput=False)
    csr_t = nc.declare_dram_parameter("csr", [NPAD, KDEG], bf16,
                                     isOutput=False)
    iota_t = nc.declare_dram_parameter("iota", [128, WDST], bf16, isOutput=False)
    ident_t = nc.declare_dram_parameter("ident", [128, 128], bf16,
                                        isOutput=False)
    w0_t = nc.declare_dram_parameter("w0", [C, C], f32, isOutput=False)
    w0t_t = nc.declare_dram_parameter("w0t", [C, C], f32, isOutput=False)
    wiht_t = nc.declare_dram_parameter("wiht", [C, 3 * C], f32, isOutput=False)
    whht_t = nc.declare_dram_parameter("whht", [C, 3 * C], f32, isOutput=False)
    bih_t = nc.declare_dram_parameter("bih", [3 * C], f32, isOutput=False)
    bhh_t = nc.declare_dram_parameter("bhh", [3 * C], f32, isOutput=False)
    linw_t = nc.declare_dram_parameter("linw", [C, 1], f32, isOutput=False)
    linb_t = nc.declare_dram_parameter("linb", [1, 1], f32, isOutput=False)
    out_t = nc.declare_dram_parameter("out", [128, NTILE], f32,
                                     isOutput=True)

    xs_paired_hbm = nc.dram_tensor("xs_paired_hbm", [HALFL, 2 * C], bf16)
    xs_all = nc.dram_tensor("xs_all", [TROWS, 2 * C], bf16,
                            addr_space="Shared")

    def bcast_partitions(ap, parts=128):
        return bass.AP(tensor=ap.tensor, offset=ap.offset,
                       ap=[[0, parts]] + list(ap.ap))

    with tile.TileContext(nc) as tc:
        with (
            tc.tile_pool(name="singles", bufs=1) as singles,
            tc.tile_pool(name="gru", bufs=1) as gru,
            tc.tile_pool(name="gpool", bufs=6) as gpool,
            tc.tile_pool(name="spool", bufs=64) as spool,
            tc.tile_pool(name="upool", bufs=3) as upool,
            tc.tile_pool(name="rpool", bufs=3) as rpool,
            tc.tile_pool(name="pagg", bufs=2, space="PSUM") as pagg,
            tc.tile_pool(name="ph", bufs=2, space="PSUM") as ph,
            tc.tile_pool(name="py", bufs=2, space="PSUM") as py,
        ):
            nhalf = NTILE // 2
            # ------------ phase A: deg -> dinv -> xs -> allgather --------
            # S-build metadata first (small; unblocks the pre-baked DVE
            # stream), then the critical csr (degree) and pre-paired x loads.
            iota_sb = singles.tile([128, WDST], bf16)
            nc.sync.dma_start(iota_sb[:], iota_t[:])
            ident_sb = singles.tile([128, 128], bf16)
            nc.sync.dma_start(ident_sb[:], ident_t[:])
            offv_sb = singles.tile([128, TOT], f32)
            nc.sync.dma_start(offv_sb[:], offv_t[:])
            ewv_sb = singles.tile([128, TOT], f32)
            nc.sync.dma_start(ewv_sb[:], ewv_t[:])

            tsplit = 13
            csr_sb = singles.tile([128, NTILE, KDEG], bf16)
            csr_r = csr_t[:].rearrange("(t r) k -> r t k", r=128)
            nc.sync.dma_start(csr_sb[:, :nhalf, :], csr_r[:, :nhalf, :])
            nc.sync.dma_start(csr_sb[:, nhalf:, :], csr_r[:, nhalf:, :])
            x2_sb = singles.tile([128, nhalf, 2 * C], bf16)
            x2_r = x_own_t[:].rearrange("(t r) f -> r t f", r=128)
            for lo, hi in ((0, tsplit), (tsplit, nhalf)):
                nc.sync.dma_start(x2_sb[:, lo:hi, :], x2_r[:, lo:hi, :])
            deg_sb = singles.tile([128, NTILE], f32)
            sqrt_sb = singles.tile([128, NTILE], f32)
            dinv_sb = singles.tile([128, NTILE], f32)
            with tc.high_priority():
                for t in range(NTILE):
                    nc.vector.reduce_sum(deg_sb[:, t:t + 1], csr_sb[:, t, :],
                                         axis=mybir.AxisListType.X)
                # sqrt(deg + 1): the +1 is the gcn_norm self-loop weight
                nc.scalar.activation(sqrt_sb[:], deg_sb[:], AF.Sqrt, bias=1.0)
                nc.vector.reciprocal(dinv_sb[:], sqrt_sb[:])

            # paired layout in SBUF: xs2[:, t, 0:C] = local row block t
            # scaled, xs2[:, t, C:2C] = block t + nhalf, so each stored
            # (partition, tile) run is 512B and the store avoids the
            # small-transfer penalty. Table row r holds rows r and r + HALFL.
            xs2_sb = singles.tile([128, nhalf, 2 * C], bf16)
            xsp_r = xs_paired_hbm[:].rearrange("(t r) f -> r t f", r=128)
            with tc.high_priority():
                for t0, t1 in ((0, tsplit), (tsplit, nhalf)):
                    for t in range(t0, t1):
                        nc.vector.tensor_scalar(
                            out=xs2_sb[:, t, :C], in0=x2_sb[:, t, :C],
                            scalar1=dinv_sb[:, t:t + 1], scalar2=None,
                            op0=OP.mult)
                        nc.vector.tensor_scalar(
                            out=xs2_sb[:, t, C:], in0=x2_sb[:, t, C:],
                            scalar1=dinv_sb[:, t + nhalf:t + nhalf + 1],
                            scalar2=None, op0=OP.mult)
                    nc.sync.dma_start(xsp_r[:, t0:t1, :], xs2_sb[:, t0:t1, :])

            linw_sb = singles.tile([C, 1], f32)
            nc.gpsimd.dma_start(out=linw_sb[:], in_=linw_t[:])
            linw_bf = singles.tile([C, 1], bf16)
            with tc.tile_wait_until(0.018):
                nc.scalar.activation(linw_bf[:], linw_sb[:], AF.Copy)
            linb_bc = singles.tile([128, 1], f32)
            nc.gpsimd.dma_start(out=linb_bc[:],
                                in_=bcast_partitions(linb_t[:1, :1]))
            if skip_collective:
                half1 = tsplit * 128
                nc.sync.dma_start(xs_all[:half1, :], xs_paired_hbm[:half1, :])
                nc.sync.dma_start(xs_all[half1:HALFL, :],
                                  xs_paired_hbm[half1:, :])
            else:
                nc.gpsimd.collective_compute(
                    "AllGather",
                    OP.bypass,
                    replica_groups=[list(range(NCORES))],
                    ins=[xs_paired_hbm[:].opt()],
                    outs=[xs_all[:].opt()],
                )
            idxlo_sb = singles.tile([128, CL], i16)
            nc.sync.dma_start(idxlo_sb[:], idxlo_t[:])
            idxhi_sb = singles.tile([128, CH], i16)
            nc.sync.dma_start(idxhi_sb[:], idxhi_t[:])

            # ---------------- GRU weight evolution ----------------------
            w0_sb = gru.tile([C, C], f32)
            nc.gpsimd.dma_start(out=w0_sb[:], in_=w0_t[:])
            w0t_sb = gru.tile([C, C], f32)
            nc.gpsimd.dma_start(out=w0t_sb[:], in_=w0t_t[:])
            wiht_sb = gru.tile([C, 3 * C], f32)
            nc.gpsimd.dma_start(out=wiht_sb[:], in_=wiht_t[:])
            whht_sb = gru.tile([C, 3 * C], f32)
            nc.gpsimd.dma_start(out=whht_sb[:], in_=whht_t[:])
            bihb_sb = gru.tile([128, 3 * C], f32)
            nc.gpsimd.dma_start(out=bihb_sb[:], in_=bcast_partitions(bih_t[:]))
            bhhb_sb = gru.tile([128, 3 * C], f32)
            nc.gpsimd.dma_start(out=bhhb_sb[:], in_=bcast_partitions(bhh_t[:]))

            gx_ps = pagg.tile([128, BLK], f32, tag="agg_ps")
            gru_defer = tc.tile_wait_until(0.018)
            gru_defer.__enter__()
            nc.tensor.matmul(gx_ps[:, :3 * C], lhsT=w0t_sb[:], rhs=wiht_sb[:],
                             start=True, stop=True)
            gxb = gru.tile([128, 3 * C], f32)
            nc.vector.tensor_tensor(out=gxb[:], in0=gx_ps[:, :3 * C],
                                    in1=bihb_sb[:], op=OP.add)
            gh_ps = pagg.tile([128, BLK], f32, tag="agg_ps")
            nc.tensor.matmul(gh_ps[:, :3 * C], lhsT=w0t_sb[:], rhs=whht_sb[:],
                             start=True, stop=True)
            ghb = gru.tile([128, 3 * C], f32)
            nc.vector.tensor_tensor(out=ghb[:], in0=gh_ps[:, :3 * C],
                                    in1=bhhb_sb[:], op=OP.add)
            rz_in = gru.tile([128, 2 * C], f32)
            nc.vector.tensor_tensor(out=rz_in[:], in0=gxb[:, :2 * C],
                                    in1=ghb[:, :2 * C], op=OP.add)
            rz = gru.tile([128, 2 * C], f32)
            nc.scalar.activation(rz[:], rz_in[:], AF.Sigmoid)
            t1 = gru.tile([128, C], f32)
            nc.vector.tensor_tensor(out=t1[:], in0=rz[:, :C],
                                    in1=ghb[:, 2 * C:], op=OP.mult)
            t2 = gru.tile([128, C], f32)
            nc.vector.tensor_tensor(out=t2[:], in0=gxb[:, 2 * C:], in1=t1[:],
                                    op=OP.add)
            n_sb = gru.tile([128, C], f32)
            nc.scalar.activation(n_sb[:], t2[:], AF.Tanh)
            d_sb = gru.tile([128, C], f32)
            nc.vector.tensor_tensor(out=d_sb[:], in0=w0_sb[:], in1=n_sb[:],
                                    op=OP.subtract)
            e_sb = gru.tile([128, C], f32)
            nc.vector.tensor_tensor(out=e_sb[:], in0=rz[:, C:], in1=d_sb[:],
                                    op=OP.mult)
            w_sb = gru.tile([C, C], f32)
            nc.vector.tensor_tensor(out=w_sb[:], in0=n_sb[:], in1=e_sb[:],
                                    op=OP.add)
            w_bf = gru.tile([C, C], bf16)
            nc.scalar.activation(w_bf[:], w_sb[:], AF.Copy)
            gru_defer.__exit__(None, None, None)

            # output accumulator (one dinv-scaled column per window)
            y_sb = singles.tile([128, NTILE], f32)

            # ---------------- phase B: gather + aggregate ----------------
            blk_list = [bb for _ in range(rep) for bb in range(nblk)]
            gtiles = {}

            def issue_block_loads(bi):
                b = blk_list[bi]
                nw = nw_b[b]
                ncol = nw * (KL + KH)
                g_sb = gpool.tile([128, MAXCOL, C], bf16, tag="g")
                nc.gpsimd.dma_gather(
                    g_sb[:, :nw * KL, :],
                    xs_all[:, :C],
                    idxlo_sb[:, lo_starts[b]:lo_starts[b] + nw * KL * 8],
                    nw * KL * 128,
                    nw * KL * 128,
                    C,
                    elem_step=2 * C,
                    single_packet=False,
                    queue_num=(2 * b) % nqueues,
                )
                nc.gpsimd.dma_gather(
                    g_sb[:, nw * KL:ncol, :],
                    xs_all[:, C:],
                    idxhi_sb[:, hi_starts[b]:hi_starts[b] + nw * KH * 8],
                    nw * KH * 128,
                    nw * KH * 128,
                    C,
                    elem_step=2 * C,
                    single_packet=False,
                    queue_num=(2 * b + 1) % nqueues,
                )
                gtiles[bi] = g_sb

            for pf in range(min(5, len(blk_list))):
                issue_block_loads(pf)
            wbase = [0]
            for nwv in nw_b:
                wbase.append(wbase[-1] + nwv)
            for bi, b in enumerate(blk_list):
                nw = nw_b[b]
                c0 = blk_start[b]
                nd = nw * WDST
                w0_b = wbase[b]
                g_sb = gtiles.pop(bi)
                if bi + 5 < len(blk_list):
                    issue_block_loads(bi + 5)

                agg_ps = pagg.tile([128, BLK], f32, tag="agg_ps")
                for w in range(nw):
                    gw = w0_b + w
                    xst = (xs2_sb[:, gw, :C] if gw < NTILE // 2
                           else xs2_sb[:, gw - NTILE // 2, C:])
                    # self-loop: + xs[d] via identity (gcn_norm weight 1)
                    nc.tensor.matmul(
                        agg_ps[:, w * WDST:(w + 1) * WDST],
                        lhsT=xst,
                        rhs=ident_sb[:],
                        start=True, stop=False,
                    )
                    for h, KX in ((0, KL), (1, KH)):
                        for j in range(KX):
                            col = (w * KL + j) if h == 0 else (
                                nw * KL + w * KH + j)
                            s_sb = spool.tile([128, WDST], bf16, tag="s")
                            nc.vector.tensor_scalar(
                                out=s_sb[:],
                                in0=iota_sb[:],
                                scalar1=offv_sb[:, c0 + col:c0 + col + 1],
                                scalar2=ewv_sb[:, c0 + col:c0 + col + 1],
                                op0=OP.is_equal,
                                op1=OP.mult,
                            )
                            nc.tensor.matmul(
                                agg_ps[:, w * WDST:(w + 1) * WDST],
                                lhsT=g_sb[:, col, :],
                                rhs=s_sb[:],
                                start=False,
                                stop=(h == 1 and j == KH - 1),
                            )

                u_sb = upool.tile([128, BLK], bf16, tag="u")
                nc.scalar.activation(u_sb[:, :nd], agg_ps[:, :nd], AF.Copy)
                h_ps = ph.tile([128, BLK], f32, tag="h")
                nc.tensor.matmul(h_ps[:, :nd], lhsT=w_bf[:], rhs=u_sb[:, :nd],
                                 start=True, stop=True)
                r_sb = rpool.tile([128, BLK], bf16, tag="r")
                nc.scalar.activation(r_sb[:, :nd], h_ps[:, :nd], AF.Relu)
                for w in range(nw):
                    gw = w0_b + w
                    y_ps = py.tile([128, 1], f32, tag="y")
                    nc.tensor.matmul(y_ps[:, :1],
                                     lhsT=r_sb[:, w * WDST:(w + 1) * WDST],
                                     rhs=linw_bf[:], start=True, stop=True)
                    nc.vector.tensor_scalar(
                        out=y_sb[:, gw:gw + 1], in0=y_ps[:, :1],
                        scalar1=dinv_sb[:, gw:gw + 1],
                        scalar2=linb_bc[:, :1],
                        op0=OP.mult, op1=OP.add)
                nc.sync.dma_start(out_t[:, w0_b:w0_b + nw],
                                  y_sb[:, w0_b:w0_b + nw])
    nc.compile()
    return nc


# ---------------------------------------------------------------------------
# Entry point
# ---------------------------------------------------------------------------

_PROG_CACHE = {}


def kernel(x, edge_index, edge_weight, W0, gru_w_ih, gru_w_hh,
           gru_b_ih, gru_b_hh, lin_w, lin_b):
    from concourse.bass_utils import run_bass_kernel_spmd

    pre, metas, L = preprocess(np.asarray(edge_index), np.asarray(edge_weight))
    key = (pre["KL"], pre["KH"], pre["KDEG"], pre["TOT"])
    if key not in _PROG_CACHE:
        _PROG_CACHE[key] = build_program(pre)
    nc = _PROG_CACHE[key]
    inp = dict(x=x, W0=W0, gru_w_ih=gru_w_ih, gru_w_hh=gru_w_hh,
               gru_b_ih=gru_b_ih, gru_b_hh=gru_b_hh, lin_w=lin_w, lin_b=lin_b)
    in_maps = make_in_maps(inp, pre, metas, L)
    res = run_bass_kernel_spmd(nc, in_maps, list(range(NCORES)))
    y = np.stack([np.asarray(res.results[c]["out"]) for c in range(NCORES)])
    nodes = np.arange(N_NODES)
    out = y[nodes // NPC, L[nodes] % 128, L[nodes] // 128]
    return out.reshape(N_NODES, 1).astype(np.float32)

